# revision 1
# baseline (speedup 1.0000x reference)
"""Trainium2 Bass kernel for AdaptiveSpikingAttention.

Strategy (8 NeuronCores, no collectives):
  - core c handles batch b = c//2, head-group hg = c%2 (4 of 8 heads).
  - The LIF recurrence has constant per-token input, so spikes are a
    function of the projection value alone.  Recurrence runs in a rescaled
    domain: u_t = vm_t * beta^-t, giving a one-op fused update
    u += d_t * q (scalar_tensor_tensor) plus a threshold compare and a
    reset, all fp32 (16-bit state flips too many spikes).
  - Tokens are sorted by window length T_i (host-side, descending) so all
    per-step work shrinks to the alive prefix A_t; masking stays exact via
    device-side compares of the gate value against the step index.
  - Spike planes are bf16 0/1 (exact) -> TensorE does the score
    contraction at full bf16 rate with K=64 row-pairs packed via
    tile_position.
  - Softmax without max-subtraction (max |score*scale| ~ 40); sums via an
    all-20s ones-matmul (folds the v_mean /20), reciprocal on DVE,
    attention+AV+Wo in bf16.
  - Host gathers: out[b] = (core 2b + core 2b+1 partials)[inv-perm] + bo.

Implementation notes:
  - The LIF runs on VectorE with a runtime-registered custom DVE op
    LIF_UPD_ANT (u_t = q*d_t + u*(u < th_prev), folding the previous
    step's reset into the update so u stays stored un-reset); spikes are
    2x-rate tensor_scalar compares (per-token v-thresholds carry the
    window mask exactly), with GPSIMD applying q/k mask multiplies in
    place and accumulating the v spike counts.
  - All fp32 inputs ride in one packed DMA tile per K-chunk so matmuls
    carry a single DMA wait; bass_rust move_matmul_waits_to_ldweights and
    generate_event_semaphores passes split any remaining multi-waits
    (hardware allows one sync wait per compute instruction).
  - One shared 8-bank PSUM pool: projections use the banks before the 8
    score accumulation groups grab them for the whole LIF, then sums/AV/Wo
    rotate through as scores free up.
"""

import math
import os

import numpy as np

B, S, E, H = 4, 512, 512, 8
Hd = E // H
HPC = 4            # heads per core
D = HPC * Hd       # 256 output dims per core
NCORES = 8
T_MAX = 20
BIGF = np.float32(3.0e38)

# packed fp32 input column layout (thv/comb/ones only)
C_THV = 0
C_COMB = 32
C_ONES = 544
PACKW = 672
# packed fp16 columns: x and the qkv weights as fp16 hi/lo split pairs
H_XHI, H_XLO = 0, 512
H_WQHI, H_WQLO = 1024, 1280
H_WKHI, H_WKLO = 1536, 1792
H_WVHI, H_WVLO = 2048, 2304
PACKH = 2560

_ALPHA = np.float64(np.exp(np.float64(-1.0 / 5.0)))
_BETA = np.float64(np.exp(np.float64(-1.0 / 20.0)))

last_exec_ns = None          # filled by kernel() when tracing
last_results = None


def _coeffs(tsteps):
    c = np.array([(1.0 - _ALPHA ** t) / (1.0 - _ALPHA) for t in range(1, tsteps + 1)])
    bp = _BETA ** np.arange(1, tsteps + 1)
    d = (c / bp).astype(np.float32)
    th = (1.0 / bp).astype(np.float32)
    return d, th


def _host_comb20(x, g1, gb1, g2, gb2, g3, gb3, c1, cb1, c2, cb2):
    """fp32 mimicry of the reference gate computation -> comb20 [B, S]."""
    f = np.float32
    x = x.astype(f)

    def sig(z):
        return (1.0 / (1.0 + np.exp(-z.astype(np.float64)))).astype(f)

    h1 = np.maximum(x @ g1 + gb1, f(0)).astype(f)
    h2 = np.maximum(h1 @ g2 + gb2, f(0)).astype(f)
    gate = sig(h2 @ g3 + gb3)
    k1 = np.maximum(x @ c1 + cb1, f(0)).astype(f)
    comp = sig(k1 @ c2 + cb2)
    comb = (f(0.7) * gate + f(0.3) * comp)[..., 0] * f(20.0)
    return comb.astype(f)


def _ceil(a, m):
    return int(-(-a // m)) * m


_BUILD_CACHE = {}
_LIF_OP = None
_VACC_OP = None


def _lif_custom_op():
    """Fused LIF update with the previous step's reset folded in:
    out = in0*s0 + in1*(in1 < s1).  Registered once per process."""
    global _LIF_OP
    if _LIF_OP is not None:
        return _LIF_OP
    import numpy as np
    from concourse.dve_spec import Spec, Src0, Src1, C0, C1, lower
    from concourse import dve_ops
    from concourse.dve_uop import DveOpSpec

    spec = Spec(
        body=Src0 * C0 + Src1 * (Src1 < C1),
        reference=lambda in0, in1, s0, s1, imm2:
            (in0 * s0 + in1 * (in1 < s1)).astype(np.float32),
    )
    def _reg(name, spec):
        if name not in dve_ops._SUB_OPCODE_FOR_NAME:
            opcode = dve_ops._CUSTOM_DVE_ROW_BASE + len(dve_ops.OPS)
            shas = {}
            for ver in ("v3", "v4"):
                try:
                    tmp = DveOpSpec(name=name, opcode=opcode,
                                    uops=lower(spec, ver=ver), rd1_en=True)
                    shas[ver] = tmp.sha(ver)
                except Exception:
                    pass
            op = dve_ops.DveOp(name, spec, subdim=False, uops_sha=shas)
            dve_ops.OPS.append(op)
            dve_ops._SUB_OPCODE_FOR_NAME[name] = opcode
            dve_ops.CUSTOM_DVE_SPECS[name] = spec
            return op
        return next(o for o in dve_ops.OPS if o.name == name)

    global _VACC_OP
    _LIF_OP = _reg("LIF_UPD_ANT", spec)
    vspec = Spec(
        body=Src1 + (Src0 >= C0),
        reference=lambda in0, in1, s0, s1, imm2:
            (in1 + (in0 >= s0)).astype(np.float32),
    )
    _VACC_OP = _reg("LIF_VACC_ANT", vspec)
    return _LIF_OP


def _build(key):
    """Build the Bass program. key = (tsteps, tuple(A), tuple(mask_needed))."""
    import concourse.bass as bass
    import concourse.bacc as bacc_mod
    import concourse.mybir as mybir
    from concourse.tile import TileContext

    tsteps, A, mask_needed = key[0], list(key[1]), list(key[2])
    f32 = mybir.dt.float32
    bf16 = mybir.dt.bfloat16
    fp16 = mybir.dt.float16
    Op = mybir.AluOpType
    AF = mybir.ActivationFunctionType
    dco, thco = _coeffs(tsteps)

    A8 = [min(S, _ceil(a, 8)) for a in A]        # update/reset range
    WR = [min(S, _ceil(a, 128)) for a in A]      # plane write range
    NB = [(a + 127) // 128 for a in A]           # alive 128-blocks
    nlive = sum(1 for a in A if a > 0)

    nc = bass.Bass()
    packf_d = nc.declare_dram_parameter("packf", [E, PACKW], f32, isOutput=False)
    packh_d = nc.declare_dram_parameter("packh", [E, PACKH], fp16, isOutput=False)
    packb_d = nc.declare_dram_parameter("packb", [64, 2048], bf16, isOutput=False)
    out_d = nc.declare_dram_parameter("out", [S, E], f32, isOutput=True)

    with TileContext(nc) as tc:
        with tc.tile_pool(name="persist", bufs=1) as P, \
             tc.tile_pool(name="psall", bufs=8, space="PSUM") as PS:
            PM = PV = PA = PS

            # ---------------- DMA inputs ----------------
            ph = []
            for i in range(4):
                t_ = P.tile([128, PACKH], fp16, tag=f"ph{i}", name=f"ph{i}")
                # q/k-critical columns first (x + wq + wk), v weights second
                nc.sync.dma_start(out=t_[:, :H_WVHI],
                                  in_=packh_d[128 * i:128 * (i + 1), :H_WVHI])
                ph.append(t_)
            for i in range(4):
                nc.sync.dma_start(out=ph[i][:, H_WVHI:],
                                  in_=packh_d[128 * i:128 * (i + 1), H_WVHI:])
            pk = []
            for i in range(4):
                t_ = P.tile([128, PACKW], f32, tag=f"pk{i}", name=f"pk{i}")
                nc.sync.dma_start(out=t_[:, :], in_=packf_d[128 * i:128 * (i + 1), :])
                pk.append(t_)
            pkb = P.tile([64, 2048], bf16, tag="pkb", name="pkb")
            nc.sync.dma_start(out=pkb[:, :], in_=packb_d[:, :])

            thv = [pk[i][:, C_THV:C_THV + tsteps] for i in range(4)]
            comb_row = pk[0][0:1, C_COMB:C_COMB + S]
            ones_row = pk[0][0:1, C_ONES:C_ONES + 128]

            # wo pre-copy on DVE so Wo matmuls see a DVE-written rhs
            wod = P.tile([64, 2048], bf16, tag="wod", name="wod")
            nc.vector.tensor_copy(out=wod[:, :], in_=pkb[:, :])
            wo = [wod[:, 512 * h:512 * (h + 1)] for h in range(4)]

            # all-20s tile for the exp-sum matmul (ACT-written)
            sumw = P.tile([128, 128], bf16, tag="sumw", name="sumw")
            nc.vector.memset(sumw[:, :], 20.0)
            sumw_a = P.tile([128, 128], bf16, tag="sumw_a", name="sumw_a")
            nc.scalar.copy(out=sumw_a[:, :], in_=sumw[:, :])

            # masks m_t (fp32 0/1), one copy per LIF engine so no spike op
            # ever needs waits from two engines (walrus 1-wait limit)
            mb = {}

            def get_mb(t):
                if t not in mb:
                    w = WR[t]
                    mbt = P.tile([128, w], bf16, tag=f"mb{t}", name=f"mb{t}")
                    nc.vector.tensor_scalar(out=mbt[:, :], in0=combbc[:, :w],
                                            scalar1=float(t), scalar2=None,
                                            op0=Op.is_gt)
                    mb[t] = mbt
                return mb[t]

            # pre-touch pack tiles from GPSIMD so later Pool ops see the DMA
            # via their own engine clock (keeps every Pool op at <=1 wait)

            # ---------------- projections (3-term bf16 split) ----------------
            # q|k packed per row: q in cols [0:S], k in cols [S:2S]
            qkT = [P.tile([128, 2 * S], f32, tag=f"qkT{r}", name=f"qkT{r}")
                   for r in range(2)]
            for r in range(2):
                for off, whi, wlo in ((0, H_WQHI, H_WQLO), (S, H_WKHI, H_WKLO)):
                    ps = PM.tile([128, S], f32, tag="ps", name="ps")
                    i_mm = 0
                    for wcol, xcol in ((whi, H_XHI), (wlo, H_XHI), (whi, H_XLO)):
                        for kc in range(4):
                            nc.tensor.matmul(
                                out=ps[:, :],
                                lhsT=ph[kc][:, wcol + 128 * r:wcol + 128 * (r + 1)],
                                rhs=ph[kc][:, xcol:xcol + S],
                                start=(i_mm == 0), stop=(i_mm == 11))
                            i_mm += 1
                    nc.scalar.copy(out=qkT[r][:, off:off + S], in_=ps[:, :])
            vTp = P.tile([128, 4 * D], f32, tag="vTp", name="vTp")
            for sb_i in range(4):
                ps = PV.tile([128, D], f32, tag="ps", name="ps")
                i_mm = 0
                for xcol, wcol in ((H_XHI, H_WVHI), (H_XHI, H_WVLO), (H_XLO, H_WVHI)):
                    for kc in range(4):
                        nc.tensor.matmul(
                            out=ps[:, :],
                            lhsT=ph[kc][:, xcol + 128 * sb_i:xcol + 128 * (sb_i + 1)],
                            rhs=ph[kc][:, wcol:wcol + D],
                            start=(i_mm == 0), stop=(i_mm == 11))
                        i_mm += 1
                nc.scalar.copy(out=vTp[:, D * sb_i:D * (sb_i + 1)], in_=ps[:, :])

            # ---------------- broadcast comb row to 128 partitions ----------------
            cb_ps = PM.tile([128, S], f32, tag="ps", name="ps")
            nc.tensor.matmul(out=cb_ps[:, :], lhsT=ones_row, rhs=comb_row,
                             start=True, stop=True)
            combbc = P.tile([128, S], f32, tag="combbc", name="combbc")
            nc.scalar.copy(out=combbc[:, :], in_=cb_ps[:, :])

            # ---------------- LIF ----------------
            # Row split: head-row r=0 (heads 0,1) entirely on DVE, r=1
            # (heads 2,3) entirely on GPSIMD; v blocks 0,1 on DVE, 2,3 on
            # GPSIMD.  Each engine owns its state, masks and planes
            # end-to-end, so ops only carry same-engine (implicit/self)
            # deps plus at most one cross-engine wait.
            LIF = _lif_custom_op()
            u_qk = [P.tile([128, 2 * S], f32, tag=f"uqk{r}", name=f"uqk{r}")
                    for r in range(2)]
            u_v = P.tile([128, 4 * D], f32, tag="uv", name="uv")
            vsum = P.tile([128, 4 * D], bf16, tag="vs", name="vs")
            vscr = [P.tile([128, 4 * D], bf16, tag=f"vscr{i}", name=f"vscr{i}")
                    for i in range(4)]
            for r in range(2):
                nc.gpsimd.memset(u_qk[r][:, :], 0.0)
            nc.gpsimd.memset(u_v[:, :], 0.0)
            nc.gpsimd.memset(vsum[:, :], 0.0)
            # plane pair tiles: q spikes in [0:w], k spikes in [S:S+w]
            qkpl = [[None, None] for _ in range(tsteps)]
            prezeroed = set()
            for t in range(tsteps):
                if mask_needed[t] and 0 < A[t] and A8[t] < WR[t]:
                    for r in range(2):
                        tag = f"qkpl{t}_{r}"
                        p_ = P.tile([128, S + WR[t]], bf16, tag=tag, name=tag)
                        nc.gpsimd.memset(p_[:, :], 0.0)
                        qkpl[t][r] = p_
                        prezeroed.add((t, r))

            def lif_step(u, r, t):
                # fused update + spike over the packed q|k tile; one wide op
                # when full-width, split ranges otherwise
                a8, w = A8[t], WR[t]
                dt_ = float(dco[t])
                tht_ = float(thco[t])
                thp_ = float(thco[t - 1]) if t > 0 else 1.0
                if a8 == S and t >= 2:
                    nc.vector._custom_dve(LIF, out=u[:, :2 * S],
                                          in0=qkT[r][:, :2 * S],
                                          in1=u[:, :2 * S], s0=dt_, s1=thp_)
                else:
                    nc.vector._custom_dve(LIF, out=u[:, :a8], in0=qkT[r][:, :a8],
                                          in1=u[:, :a8], s0=dt_, s1=thp_)
                    nc.vector._custom_dve(LIF, out=u[:, S:S + a8],
                                          in0=qkT[r][:, S:S + a8],
                                          in1=u[:, S:S + a8], s0=dt_, s1=thp_)
                if (t, r) in prezeroed:
                    p_ = qkpl[t][r]
                    nc.vector.tensor_scalar(out=p_[:, :a8], in0=u[:, :a8],
                                            scalar1=tht_, scalar2=None, op0=Op.is_ge)
                    nc.vector.tensor_scalar(out=p_[:, S:S + a8], in0=u[:, S:S + a8],
                                            scalar1=tht_, scalar2=None, op0=Op.is_ge)
                    nc.gpsimd.tensor_tensor(out=p_[:, :a8], in0=p_[:, :a8],
                                            in1=get_mb(t)[:, :a8], op=Op.mult)
                    nc.gpsimd.tensor_tensor(out=p_[:, S:S + a8], in0=p_[:, S:S + a8],
                                            in1=get_mb(t)[:, :a8], op=Op.mult)
                    return
                tag = f"qkpl{t}_{r}"
                p_ = P.tile([128, S + w], bf16, tag=tag, name=tag)
                if w == S and t >= 2:
                    nc.vector.tensor_scalar(out=p_[:, :2 * S], in0=u[:, :2 * S],
                                            scalar1=tht_, scalar2=None, op0=Op.is_ge)
                else:
                    nc.vector.tensor_scalar(out=p_[:, :w], in0=u[:, :w],
                                            scalar1=tht_, scalar2=None, op0=Op.is_ge)
                    nc.vector.tensor_scalar(out=p_[:, S:S + w], in0=u[:, S:S + w],
                                            scalar1=tht_, scalar2=None, op0=Op.is_ge)
                if mask_needed[t]:
                    nc.gpsimd.tensor_tensor(out=p_[:, :w], in0=p_[:, :w],
                                            in1=get_mb(t)[:, :w], op=Op.mult)
                    nc.gpsimd.tensor_tensor(out=p_[:, S:S + w], in0=p_[:, S:S + w],
                                            in1=get_mb(t)[:, :w], op=Op.mult)
                qkpl[t][r] = p_

            for r in range(2):
                for t in range(tsteps):
                    if A[t] == 0:
                        break
                    lif_step(u_qk[r], r, t)
                    if r == 1:
                        dt_ = float(dco[t])
                        thp_ = float(thco[t - 1]) if t > 0 else 1.0
                        nv = D * NB[t]
                        nc.vector._custom_dve(LIF, out=u_v[:, :nv], in0=vTp[:, :nv],
                                              in1=u_v[:, :nv], s0=dt_, s1=thp_)
                        scr = vscr[t % 4]
                        for i in range(NB[t]):
                            nc.vector.tensor_scalar(
                                out=scr[:, D * i:D * (i + 1)],
                                in0=u_v[:, D * i:D * (i + 1)],
                                scalar1=thv[i][:, t:t + 1], scalar2=None,
                                op0=Op.is_ge)
                        nc.gpsimd.tensor_tensor(out=vsum[:, :nv], in0=vsum[:, :nv],
                                                in1=scr[:, :nv], op=Op.add)

            # ---------------- scores + softmax ----------------
            lastt = [max(t for t in range(nlive) if NB[t] > jb) for jb in range(4)]
            expT = {}
            gidx = 0
            prev_exp = []
            for rp in range(2):
                for jb in (3, 2, 1, 0):     # ascending lifetime: early groups
                    ps_pair = [PS.tile([128, S], f32, tag="ps", name="ps")
                               for _ in range(2)]
                    for t in range(lastt[jb] + 1):
                        if NB[t] <= jb:
                            continue
                        w = A8[t] if t > 0 else S
                        for hh in range(2):
                            nc.tensor.matmul(
                                out=ps_pair[hh][:, :w],
                                lhsT=qkpl[t][rp][64 * hh:64 * (hh + 1),
                                                 S + 128 * jb:S + 128 * (jb + 1)],
                                rhs=qkpl[t][rp][64 * hh:64 * (hh + 1), :w],
                                start=(t == 0), stop=(t == lastt[jb]))
                    last_ex = None
                    for hh in range(2):
                        h = 2 * rp + hh
                        ex = P.tile([128, S], bf16, tag=f"exp{h}_{jb}",
                                    name=f"exp{h}_{jb}")
                        nc.scalar.activation(out=ex[:, :], in_=ps_pair[hh][:, :],
                                             func=AF.Exp, scale=float(Hd ** -0.5))
                        expT[(h, jb)] = ex
                        last_ex = ex
                    prev_exp.append(last_ex)
                    gidx += 1

            # AV on unnormalized exp; 1/(20*sum) folds into the PSUM copy
            av = []
            for h in range(4):
                # reverse j-block order: high blocks' exp/vsum finalize
                # mid-LIF, so these accumulations start before the LIF ends
                sps = PM.tile([128, S], f32, tag="ps", name="ps")
                for jb in (3, 2, 1, 0):
                    nc.tensor.matmul(out=sps[:, :], lhsT=sumw_a[:, :],
                                     rhs=expT[(h, jb)][:, :],
                                     start=(jb == 3), stop=(jb == 0))
                rec = P.tile([128, S], f32, tag=f"rec{h}", name=f"rec{h}")
                nc.vector.reciprocal_approx_fast(out=rec[:, :], in_=sps[:, :])
                ps = PA.tile([64, S], f32, tag="ps", name="ps")
                for jb in (3, 2, 1, 0):
                    nc.tensor.matmul(out=ps[:, :],
                                     lhsT=vsum[:, D * jb + 64 * h:D * jb + 64 * (h + 1)],
                                     rhs=expT[(h, jb)][:, :],
                                     start=(jb == 3), stop=(jb == 0))
                sb = P.tile([64, S], bf16, tag=f"av{h}", name=f"av{h}")
                nc.vector.tensor_tensor(out=sb[:, :], in0=ps[:, :],
                                        in1=rec[0:64, :], op=Op.mult)
                av.append(sb)

            for ib in range(4):
                ps = PA.tile([128, E], f32, tag="ps", name="ps")
                for h in range(4):
                    nc.tensor.matmul(out=ps[:, :],
                                     lhsT=av[h][:, 128 * ib:128 * (ib + 1)],
                                     rhs=wo[h],
                                     start=(h == 0), stop=(h == 3))
                osb = P.tile([128, E], f32, tag=f"osb{ib}", name=f"osb{ib}")
                nc.scalar.copy(out=osb[:, :], in_=ps[:, :])
                nc.sync.dma_start(out=out_d[128 * ib:128 * (ib + 1), :], in_=osb[:, :])

    import bass_rust as _bass_rust
    _bass_rust.move_matmul_waits_to_ldweights(nc.m)
    _bass_rust.generate_event_semaphores(nc)
    _bass_rust.codegen_inst_isa_subclasses(nc)
    return nc


def _plan(comb20):
    """Sort + alive-count plan shared by kernel() and the test harness."""
    perm = np.argsort(-comb20, axis=1, kind="stable")
    comb_sorted = np.take_along_axis(comb20, perm, axis=1)
    eps = np.float32(0.01)
    tsteps = int(min(T_MAX, max(1, math.ceil(float(comb_sorted.max() + eps)))))
    A, mask_needed = [], []
    for t in range(tsteps):
        cnt = int(max((comb_sorted[b] > t - eps).sum() for b in range(B)))
        A.append(min(S, cnt + 4) if 0 < cnt < S else cnt)
        mask_needed.append(bool((comb_sorted > t + eps).sum() < B * S))
    for t in range(tsteps - 2, -1, -1):
        A[t] = max(A[t], A[t + 1])
    A[0] = S
    return perm, comb_sorted, tsteps, A, mask_needed


def make_in_maps(inputs, perm, comb_sorted, tsteps):
    import ml_dtypes
    f = np.float32
    bf = np.dtype(ml_dtypes.bfloat16)
    x = np.asarray(inputs["x"], f)
    Wq = np.asarray(inputs["Wq"], f)
    Wk = np.asarray(inputs["Wk"], f)
    Wv = np.asarray(inputs["Wv"], f)
    Wo = np.asarray(inputs["Wo"], f)
    _, thco = _coeffs(tsteps)
    in_maps = []
    def split16(a):
        hi = a.astype(np.float16)
        lo = (a - hi.astype(f)).astype(np.float16)
        return hi, lo

    for core in range(NCORES):
        b, hg = core // 2, core % 2
        sl = slice(hg * D, (hg + 1) * D)
        cs = comb_sorted[b]
        packf = np.zeros((E, PACKW), f)
        alive = cs[:, None] > np.arange(tsteps)[None, :]
        packf[:, C_THV:C_THV + tsteps] = np.where(alive, thco[None, :], BIGF)
        packf[0, C_COMB:C_COMB + S] = cs
        packf[0, C_ONES:C_ONES + 128] = 1.0
        packh = np.zeros((E, PACKH), np.float16)
        xhi, xlo = split16(np.ascontiguousarray(x[b][perm[b]].T))
        packh[:, H_XHI:H_XHI + S] = xhi
        packh[:, H_XLO:H_XLO + S] = xlo
        for w, chi, clo in ((Wq, H_WQHI, H_WQLO), (Wk, H_WKHI, H_WKLO),
                            (Wv, H_WVHI, H_WVLO)):
            whi, wlo = split16(np.ascontiguousarray(w[:, sl]))
            packh[:, chi:chi + D] = whi
            packh[:, clo:clo + D] = wlo
        packb = np.zeros((64, 2048), f)
        for h in range(4):
            packb[:, 512 * h:512 * (h + 1)] = Wo[hg * D + 64 * h:hg * D + 64 * (h + 1), :]
        in_maps.append({"packf": packf, "packh": packh,
                        "packb": packb.astype(bf)})
    return in_maps


def kernel(**inputs):
    global last_exec_ns, last_results
    f = np.float32
    x = np.asarray(inputs["x"], f)
    bo = np.asarray(inputs["bo"], f)

    comb20 = _host_comb20(x,
                          np.asarray(inputs["g1"], f), np.asarray(inputs["gb1"], f),
                          np.asarray(inputs["g2"], f), np.asarray(inputs["gb2"], f),
                          np.asarray(inputs["g3"], f), np.asarray(inputs["gb3"], f),
                          np.asarray(inputs["c1"], f), np.asarray(inputs["cb1"], f),
                          np.asarray(inputs["c2"], f), np.asarray(inputs["cb2"], f))
    perm, comb_sorted, tsteps, A, mask_needed = _plan(comb20)

    key = (tsteps, tuple(A), tuple(mask_needed))
    if key not in _BUILD_CACHE:
        _BUILD_CACHE[key] = _build(key)
    nc = _BUILD_CACHE[key]

    in_maps = make_in_maps(inputs, perm, comb_sorted, tsteps)

    from concourse.bass_utils import run_bass_kernel_spmd
    trace = bool(int(os.environ.get("KERNEL_TRACE", "0")))
    try:
        res = run_bass_kernel_spmd(nc, in_maps, core_ids=list(range(NCORES)),
                                   trace=trace)
    except (ModuleNotFoundError, ImportError):
        res = run_bass_kernel_spmd(nc, in_maps, core_ids=list(range(NCORES)),
                                   trace=False)
    last_results = res
    last_exec_ns = res.exec_time_ns

    out = np.empty((B, S, E), np.float32)
    for b in range(B):
        inv = np.empty(S, np.int64)
        inv[perm[b]] = np.arange(S)
        part = res.results[2 * b]["out"] + res.results[2 * b + 1]["out"]
        out[b] = part[inv] + bo[None, :]
    return out



# revision 22
# speedup vs baseline: 1.2819x; 1.2819x over previous
"""Trainium2 Bass kernel for AdaptiveSpikingAttention.

Strategy (8 NeuronCores, no collectives):
  - core c handles batch b = c//2, head-group hg = c%2 (4 of 8 heads).
  - q/k LIF runs the rescaled-domain recurrence on DVE (custom fused op);
    spikes are written as fp8 0/1 planes with steps PAIRED and interleaved
    along the free axis so each score matmul contracts two timesteps at
    once in DoubleRow mode (fp8, 0.5 cycles/row).
  - v path needs only the per-token spike COUNT within the window, and the
    count is a monotone staircase in the projection value: vsum[j,d] =
    sum_m [v[j,d] >= c(m, T_j)] with a host-precomputed threshold table
    c(m,T) (bisected fp64 LIF), T_j entering via per-partition threshold
    columns.  No v recurrence on device at all.
  - Tokens are host-sorted by window length (descending); per-step work
    shrinks to the alive prefix.  Ragged per-batch masking only touches a
    narrow window [Amin, WR) per step instead of full width.
  - Softmax without max-subtraction; sums via an all-20s ones-matmul
    (folds the v_mean /20), reciprocal on DVE, attention+AV in bf16,
    Wo with head-paired K=128 matmuls.
  - Host gathers: out[b] = (core 2b + core 2b+1 partials)[inv-perm] + bo.
"""

import math
import os

import numpy as np

B, S, E, H = 4, 512, 512, 8
Hd = E // H
HPC = 4            # heads per core
D = HPC * Hd       # 256 output dims per core
NCORES = 8
T_MAX = 20
BIGF = np.float32(3.0e38)

# packed fp32 input column layout: c(m, T_j) per token block
C_THV = 0          # 4 blocks x 20 cols (BIGF where m > T_j)
PACKW = 80
# packc single-row layout: comb (512) + ones (128)
PC_COMB = 0
PC_ONES = 512
PACKC = 640
# packed fp32 columns: x^T and the qkv weight blocks (fp32r matmuls);
# v-critical columns [0:768) first so v projections can start early
R_X = 0
R_WV = 512
R_WQ = 768
R_WK = 1024
PACKR = 1280

_ALPHA = np.float64(np.exp(np.float64(-1.0 / 5.0)))
_BETA = np.float64(np.exp(np.float64(-1.0 / 20.0)))

last_exec_ns = None          # filled by kernel() when tracing
last_results = None


def _coeffs(tsteps):
    c = np.array([(1.0 - _ALPHA ** t) / (1.0 - _ALPHA) for t in range(1, tsteps + 1)])
    bp = _BETA ** np.arange(1, tsteps + 1)
    d = (c / bp).astype(np.float32)
    th = (1.0 / bp).astype(np.float32)
    return d, th


_CTAB = None


def _count_table():
    """c[m-1][T-1] = min x such that the LIF with constant input x spikes
    >= m times within T steps (fp64 bisection; BIGF where unreachable)."""
    global _CTAB
    if _CTAB is not None:
        return _CTAB

    def counts(x):
        # vectorized fp64 LIF; returns [len(x), T_MAX] cumulative counts
        x = np.asarray(x, np.float64)
        vm = np.zeros_like(x)
        isyn = np.zeros_like(x)
        cnt = np.zeros_like(x)
        out = np.empty((len(x), T_MAX))
        for t in range(T_MAX):
            isyn = _ALPHA * isyn + x
            vm = _BETA * vm + isyn
            s = vm >= 1.0
            cnt = cnt + s
            vm = np.where(s, 0.0, vm)
            out[:, t] = cnt
        return out

    tab = np.full((T_MAX, T_MAX), BIGF, np.float32)
    for T in range(1, T_MAX + 1):
        for m in range(1, T + 1):
            lo, hi = 0.0, 64.0
            if counts(np.array([hi]))[0, T - 1] < m:
                continue
            for _ in range(60):
                mid = 0.5 * (lo + hi)
                if counts(np.array([mid]))[0, T - 1] >= m:
                    hi = mid
                else:
                    lo = mid
            tab[m - 1, T - 1] = np.float32(hi)
    _CTAB = tab
    return tab


def _host_comb20(x, g1, gb1, g2, gb2, g3, gb3, c1, cb1, c2, cb2):
    """fp32 mimicry of the reference gate computation -> comb20 [B, S]."""
    f = np.float32
    x = x.astype(f)

    def sig(z):
        return (1.0 / (1.0 + np.exp(-z.astype(np.float64)))).astype(f)

    h1 = np.maximum(x @ g1 + gb1, f(0)).astype(f)
    h2 = np.maximum(h1 @ g2 + gb2, f(0)).astype(f)
    gate = sig(h2 @ g3 + gb3)
    k1 = np.maximum(x @ c1 + cb1, f(0)).astype(f)
    comp = sig(k1 @ c2 + cb2)
    comb = (f(0.7) * gate + f(0.3) * comp)[..., 0] * f(20.0)
    return comb.astype(f)


def _ceil(a, m):
    return int(-(-a // m)) * m


_BUILD_CACHE = {}
_LIF_OP = None


def _lif_custom_op():
    """Fused LIF update with the previous step's reset folded in:
    out = in0*s0 + in1*(in1 < s1).  Registered once per process."""
    global _LIF_OP
    if _LIF_OP is not None:
        return _LIF_OP
    import numpy as np
    from concourse.dve_spec import Spec, Src0, Src1, C0, C1, lower
    from concourse import dve_ops
    from concourse.dve_uop import DveOpSpec

    spec = Spec(
        body=Src0 * C0 + Src1 * (Src1 < C1),
        reference=lambda in0, in1, s0, s1, imm2:
            (in0 * s0 + in1 * (in1 < s1)).astype(np.float32),
    )
    def _reg(name, spec):
        if name not in dve_ops._SUB_OPCODE_FOR_NAME:
            opcode = dve_ops._CUSTOM_DVE_ROW_BASE + len(dve_ops.OPS)
            shas = {}
            for ver in ("v3", "v4"):
                try:
                    tmp = DveOpSpec(name=name, opcode=opcode,
                                    uops=lower(spec, ver=ver), rd1_en=True)
                    shas[ver] = tmp.sha(ver)
                except Exception:
                    pass
            op = dve_ops.DveOp(name, spec, subdim=False, uops_sha=shas)
            dve_ops.OPS.append(op)
            dve_ops._SUB_OPCODE_FOR_NAME[name] = opcode
            dve_ops.CUSTOM_DVE_SPECS[name] = spec
            return op
        return next(o for o in dve_ops.OPS if o.name == name)

    _LIF_OP = _reg("LIF_UPD_ANT", spec)
    return _LIF_OP


def _build(key):
    """Build the Bass program.
    key = (tsteps, tuple(A), tuple(mask_needed), tuple(Amin))."""
    import concourse.bass as bass
    import concourse.mybir as mybir
    from concourse.tile import TileContext

    tsteps, A, mask_needed, Amin = (key[0], list(key[1]), list(key[2]),
                                    list(key[3]))
    f32 = mybir.dt.float32
    f32r = mybir.dt.float32r
    bf16 = mybir.dt.bfloat16
    fp8 = mybir.dt.float8e4
    Op = mybir.AluOpType
    AF = mybir.ActivationFunctionType
    PM_DR = mybir.MatmulPerfMode.DoubleRow
    dco, thco = _coeffs(tsteps)

    A8 = [min(S, _ceil(a, 8)) for a in A]        # update/write range
    WR = [min(S, _ceil(a, 128)) for a in A]      # k-plane write range
    NB = [(a + 127) // 128 for a in A]           # alive 128-blocks
    LO = [min(Amin[t] // 8 * 8, A8[t]) for t in range(tsteps)]
    # v-count upper bound per token block
    maxTb = [sum(1 for t in range(tsteps) if A[t] > 128 * i) for i in range(4)]

    npair = (tsteps + 1) // 2
    pw = [min(S, _ceil(A8[2 * p], 16)) for p in range(npair)]   # q width
    kw = [WR[2 * p] for p in range(npair)]              # k written width

    nc = bass.Bass()
    packf_d = nc.declare_dram_parameter("packf", [128, PACKW], f32, isOutput=False)
    packc_d = nc.declare_dram_parameter("packc", [1, PACKC], f32, isOutput=False)
    packr_d = nc.declare_dram_parameter("packr", [E, PACKR], f32r, isOutput=False)
    packb_d = nc.declare_dram_parameter("packb", [128, 1024], bf16, isOutput=False)
    out_d = nc.declare_dram_parameter("out", [S, E], f32, isOutput=True)

    with TileContext(nc) as tc:
        with tc.tile_pool(name="persist", bufs=1) as P, \
             tc.tile_pool(name="psall", bufs=8, space="PSUM") as PS:
            PM = PV = PA = PS

            # ---------------- DMA inputs (spread across engine queues) ----
            pkf = P.tile([128, PACKW], f32, tag="pkf", name="pkf")
            nc.gpsimd.dma_start(out=pkf[:, :], in_=packf_d[:, :])
            pkc = P.tile([1, PACKC], f32, tag="pkc", name="pkc")
            nc.sync.dma_start(out=pkc[:, :], in_=packc_d[:, :])
            dmaq = [nc.sync, nc.gpsimd, nc.scalar, nc.scalar]
            pr = []
            for i in range(4):
                t_ = P.tile([128, PACKR], f32r, tag=f"pr{i}", name=f"pr{i}")
                # v-critical columns first (x + Wv), q/k weights second
                dmaq[i].dma_start(out=t_[:, :R_WQ],
                                  in_=packr_d[128 * i:128 * (i + 1), :R_WQ])
                pr.append(t_)
            for i in range(4):
                dmaq[i].dma_start(out=pr[i][:, R_WQ:],
                                  in_=packr_d[128 * i:128 * (i + 1), R_WQ:])
            pkb = P.tile([128, 1024], bf16, tag="pkb", name="pkb")
            nc.sync.dma_start(out=pkb[:, :], in_=packb_d[:, :])

            thvm = [pkf[:, 20 * i:20 * (i + 1)] for i in range(4)]
            comb_row = pkc[0:1, PC_COMB:PC_COMB + S]
            ones_row = pkc[0:1, PC_ONES:PC_ONES + 128]

            # wo pre-copy on DVE so Wo matmuls see a DVE-written rhs
            wod = P.tile([128, 1024], bf16, tag="wod", name="wod")
            nc.vector.tensor_copy(out=wod[:, :], in_=pkb[:, :])
            wo = [wod[:, 512 * hp:512 * (hp + 1)] for hp in range(2)]

            # all-20s tile for the exp-sum matmul (ACT-written); also preload
            # the ACT Exp/Reciprocal tables off the critical path
            sumw = P.tile([128, 128], bf16, tag="sumw", name="sumw")
            nc.vector.memset(sumw[:, :], 20.0)
            sumw_a = P.tile([128, 128], bf16, tag="sumw_a", name="sumw_a")
            nc.scalar.copy(out=sumw_a[:, :], in_=sumw[:, :])
            actw = P.tile([128, 8], f32, tag="actw", name="actw")
            nc.scalar.activation(out=actw[:, :], in_=sumw[:, :8],
                                 func=AF.Exp, scale=1.0)

            # ---------------- projections (fp32r) ----------------
            # v first: its consumers (count compares) fill DVE's startup gap
            vTp = P.tile([128, 4 * D], f32, tag="vTp", name="vTp")
            for sb_i in range(4):
                ps = PV.tile([128, D], f32, tag="ps", name="ps")
                for kc in range(4):
                    nc.tensor.matmul(
                        out=ps[:, :],
                        lhsT=pr[kc][:, R_X + 128 * sb_i:R_X + 128 * (sb_i + 1)],
                        rhs=pr[kc][:, R_WV:R_WV + D],
                        start=(kc == 0), stop=(kc == 3))
                nc.scalar.copy(out=vTp[:, D * sb_i:D * (sb_i + 1)], in_=ps[:, :])

            # q|k packed per row: q in cols [0:S], k in cols [S:2S]
            qkT = [P.tile([128, 2 * S], f32, tag=f"qkT{r}", name=f"qkT{r}")
                   for r in range(2)]
            for r in range(2):
                for off, wcol in ((0, R_WQ), (S, R_WK)):
                    ps = PM.tile([128, S], f32, tag="ps", name="ps")
                    for kc in range(4):
                        nc.tensor.matmul(
                            out=ps[:, :],
                            lhsT=pr[kc][:, wcol + 128 * r:wcol + 128 * (r + 1)],
                            rhs=pr[kc][:, R_X:R_X + S],
                            start=(kc == 0), stop=(kc == 3))
                    nc.scalar.copy(out=qkT[r][:, off:off + S], in_=ps[:, :])

            # ---------------- broadcast comb row to 128 partitions ----------------
            cb_ps = PM.tile([128, S], f32, tag="ps", name="ps")
            nc.tensor.matmul(out=cb_ps[:, :], lhsT=ones_row, rhs=comb_row,
                             start=True, stop=True)
            combbc = P.tile([128, S], f32, tag="combbc", name="combbc")
            nc.scalar.copy(out=combbc[:, :], in_=cb_ps[:, :])

            # ---------------- v spike counts (no recurrence) ----------------
            # vsum[j,d] = sum_m [vTp >= c(m, T_j)]; compares on DVE (2x),
            # accumulation on Pool.
            vsum = P.tile([128, 4 * D], bf16, tag="vs", name="vs")
            vscr = [P.tile([128, 4 * D], bf16, tag=f"vscr{i}", name=f"vscr{i}")
                    for i in range(2)]
            for m in range(1, maxTb[0] + 1):
                nbm = sum(1 for i in range(4) if maxTb[i] >= m)
                dst = vsum if m == 1 else vscr[m % 2]
                for i in range(nbm):
                    eng = nc.vector if i < 2 else nc.gpsimd
                    eng.tensor_scalar(
                        out=dst[:, D * i:D * (i + 1)],
                        in0=vTp[:, D * i:D * (i + 1)],
                        scalar1=thvm[i][:, m - 1:m], scalar2=None,
                        op0=Op.is_ge)
                if m > 1:
                    nc.gpsimd.tensor_tensor(
                        out=vsum[:, :D * nbm], in0=vsum[:, :D * nbm],
                        in1=dst[:, :D * nbm], op=Op.add)

            # ---------------- q/k LIF + fp8 spike planes ----------------
            LIF = _lif_custom_op()
            u_qk = [P.tile([128, 2 * S], f32, tag=f"uqk{r}", name=f"uqk{r}")
                    for r in range(2)]
            for r in range(2):
                nc.gpsimd.memset(u_qk[r][:, :], 0.0)

            # plane tiles per (pair, r): q interleaved at [0:2*pw], k
            # interleaved at [2S : 2S+2*kw]
            planes = [[None, None] for _ in range(npair)]
            for p in range(npair):
                for r in range(2):
                    tag = f"pl{p}_{r}"
                    planes[p][r] = P.tile([128, 2 * pw[p] + 2 * kw[p]], fp8,
                                          tag=tag, name=tag)

            # per-step ragged mask windows (built on DVE from combbc, fp8)
            mbw = {}

            def get_mb(t, hi):
                key2 = (t, hi)
                if key2 not in mbw:
                    lo = LO[t]
                    w = hi - lo
                    mbt = P.tile([128, w], fp8, tag=f"mb{t}_{hi}",
                                 name=f"mb{t}_{hi}")
                    nc.vector.tensor_scalar(out=mbt[:, :],
                                            in0=combbc[:, lo:hi],
                                            scalar1=float(t), scalar2=None,
                                            op0=Op.is_gt)
                    mbw[key2] = mbt
                return mbw[key2]

            for t in range(tsteps):
                if A[t] == 0:
                    break
                p = t // 2
                sub = t % 2
                wq, wk = pw[p], kw[p]
                dt_ = float(dco[t])
                tht_ = float(thco[t])
                thp_ = float(thco[t - 1]) if t > 0 else 1.0
                a8 = A8[t]
                for r in range(2):
                    u = u_qk[r]
                    # fused update + spike over the packed q|k tile; the
                    # split-range case merges q and k into one op via a
                    # 2-segment 3D AP
                    if a8 == S and t >= 2:
                        nc.vector._custom_dve(LIF, out=u[:, :2 * S],
                                              in0=qkT[r][:, :2 * S],
                                              in1=u[:, :2 * S], s0=dt_, s1=thp_)
                    elif t < 2:
                        nc.vector._custom_dve(LIF, out=u[:, :a8],
                                              in0=qkT[r][:, :a8],
                                              in1=u[:, :a8], s0=dt_, s1=thp_)
                        nc.vector._custom_dve(LIF, out=u[:, S:S + a8],
                                              in0=qkT[r][:, S:S + a8],
                                              in1=u[:, S:S + a8], s0=dt_, s1=thp_)
                    else:
                        u3 = u[:, :].rearrange("p (two j) -> p two j", two=2)
                        q3 = qkT[r][:, :].rearrange("p (two j) -> p two j", two=2)
                        nc.vector._custom_dve(LIF, out=u3[:, :, :a8],
                                              in0=q3[:, :, :a8],
                                              in1=u3[:, :, :a8], s0=dt_, s1=thp_)
                    pl = planes[p][r]
                    qo = sub * wq
                    ko = 2 * wq + sub * wk
                    nc.vector.tensor_scalar(
                        out=pl[:, qo:qo + wq], in0=u[:, :wq],
                        scalar1=tht_, scalar2=None, op0=Op.is_ge)
                    nc.vector.tensor_scalar(
                        out=pl[:, ko:ko + wk],
                        in0=u[:, S:S + wk],
                        scalar1=tht_, scalar2=None, op0=Op.is_ge)
                    # ragged per-batch masking over the narrow window
                    if LO[t] < wq:
                        mb = get_mb(t, wq)
                        nc.gpsimd.tensor_tensor(
                            out=pl[:, qo + LO[t]:qo + wq],
                            in0=pl[:, qo + LO[t]:qo + wq],
                            in1=mb[:, :], op=Op.mult)
                    if LO[t] < wk:
                        mb = get_mb(t, wk)
                        nc.gpsimd.tensor_tensor(
                            out=pl[:, ko + LO[t]:ko + wk],
                            in0=pl[:, ko + LO[t]:ko + wk],
                            in1=mb[:, :], op=Op.mult)

            # if tsteps is odd, the dangling substep of the last pair must be
            # zero so DoubleRow contraction adds nothing
            if tsteps % 2 == 1:
                p = npair - 1
                for r in range(2):
                    pl = planes[p][r]
                    nc.gpsimd.memset(pl[:, pw[p]:2 * pw[p]], 0.0)
                    nc.gpsimd.memset(
                        pl[:, 2 * pw[p] + kw[p]:2 * pw[p] + 2 * kw[p]], 0.0)

            # ---------------- scores + softmax ----------------
            lastp = [max(p for p in range(npair) if NB[2 * p] > jb)
                     for jb in range(4)]
            expT = {}
            for rp in range(2):
                for jb in (3, 2, 1, 0):     # ascending lifetime: early groups
                    ps_pair = [PS.tile([128, S], f32, tag="ps", name="ps")
                               for _ in range(2)]
                    for p in range(lastp[jb] + 1):
                        if NB[2 * p] <= jb:
                            continue
                        w = pw[p]
                        for hh in range(2):
                            pl = planes[p][rp]
                            wqp, wkp = pw[p], kw[p]
                            lhsT = pl[64 * hh:64 * (hh + 1),
                                      2 * wqp:2 * wqp + 2 * wkp]
                            lhsT = lhsT.rearrange("p (two j) -> p two j", two=2)
                            lhsT = lhsT[:, :, 128 * jb:128 * (jb + 1)]
                            rhs = pl[64 * hh:64 * (hh + 1), :2 * wqp]
                            rhs = rhs.rearrange("p (two j) -> p two j", two=2)
                            rhs = rhs[:, :, :w]
                            nc.tensor.matmul(
                                out=ps_pair[hh][:, :w],
                                lhsT=lhsT, rhs=rhs,
                                perf_mode=PM_DR,
                                start=(p == 0), stop=(p == lastp[jb]))
                    for hh in range(2):
                        h = 2 * rp + hh
                        ex = P.tile([128, S], bf16, tag=f"exp{h}_{jb}",
                                    name=f"exp{h}_{jb}")
                        nc.scalar.activation(out=ex[:, :], in_=ps_pair[hh][:, :],
                                             func=AF.Exp, scale=float(Hd ** -0.5))
                        expT[(h, jb)] = ex

            # AV on unnormalized exp; 1/(20*sum) folds into the PSUM copy
            av = [P.tile([128, S], bf16, tag=f"av{hp}", name=f"av{hp}")
                  for hp in range(2)]
            for h in range(4):
                # reverse j-block order: high blocks' exp/vsum finalize
                # early, so these accumulations start before the LIF ends
                sps = PM.tile([128, S], f32, tag="ps", name="ps")
                for jb in (3, 2, 1, 0):
                    nc.tensor.matmul(out=sps[:, :], lhsT=sumw_a[:, :],
                                     rhs=expT[(h, jb)][:, :],
                                     start=(jb == 3), stop=(jb == 0))
                rec = P.tile([128, S], f32, tag=f"rec{h}", name=f"rec{h}")
                nc.vector.reciprocal_approx_fast(out=rec[:, :], in_=sps[:, :])
                ps = PA.tile([64, S], f32, tag="ps", name="ps")
                for jb in (3, 2, 1, 0):
                    nc.tensor.matmul(out=ps[:, :],
                                     lhsT=vsum[:, D * jb + 64 * h:D * jb + 64 * (h + 1)],
                                     rhs=expT[(h, jb)][:, :],
                                     start=(jb == 3), stop=(jb == 0))
                hp, sub = h // 2, h % 2
                nc.vector.tensor_tensor(out=av[hp][64 * sub:64 * (sub + 1), :],
                                        in0=ps[:, :],
                                        in1=rec[0:64, :], op=Op.mult)

            outq = [nc.sync, nc.gpsimd, nc.scalar, nc.sync]
            for ib in range(4):
                ps = PA.tile([128, E], f32, tag="ps", name="ps")
                for hp in range(2):
                    nc.tensor.matmul(out=ps[:, :],
                                     lhsT=av[hp][:, 128 * ib:128 * (ib + 1)],
                                     rhs=wo[hp],
                                     start=(hp == 0), stop=(hp == 1))
                osb = P.tile([128, E], f32, tag=f"osb{ib}", name=f"osb{ib}")
                if ib % 2 == 0:
                    nc.scalar.copy(out=osb[:, :], in_=ps[:, :])
                else:
                    nc.vector.tensor_copy(out=osb[:, :], in_=ps[:, :])
                outq[ib].dma_start(out=out_d[128 * ib:128 * (ib + 1), :],
                                   in_=osb[:, :])

    import bass_rust as _bass_rust
    _bass_rust.move_matmul_waits_to_ldweights(nc.m)
    _bass_rust.generate_event_semaphores(nc)
    _bass_rust.codegen_inst_isa_subclasses(nc)
    return nc


def _plan(comb20):
    """Sort + alive-count plan shared by kernel() and the test harness."""
    perm = np.argsort(-comb20, axis=1, kind="stable")
    comb_sorted = np.take_along_axis(comb20, perm, axis=1)
    eps = np.float32(0.01)
    tsteps = int(min(T_MAX, max(1, math.ceil(float(comb_sorted.max() + eps)))))
    A, mask_needed, Amin = [], [], []
    for t in range(tsteps):
        cnt = int(max((comb_sorted[b] > t - eps).sum() for b in range(B)))
        A.append(min(S, cnt + 4) if 0 < cnt < S else cnt)
        mask_needed.append(bool((comb_sorted > t + eps).sum() < B * S))
        Amin.append(int(min((comb_sorted[b] > t + eps).sum() for b in range(B))))
    for t in range(tsteps - 2, -1, -1):
        A[t] = max(A[t], A[t + 1])
    A[0] = S
    return perm, comb_sorted, tsteps, A, mask_needed, Amin


def make_in_maps(inputs, perm, comb_sorted, tsteps):
    import ml_dtypes
    f = np.float32
    bf = np.dtype(ml_dtypes.bfloat16)
    x = np.asarray(inputs["x"], f)
    Wq = np.asarray(inputs["Wq"], f)
    Wk = np.asarray(inputs["Wk"], f)
    Wv = np.asarray(inputs["Wv"], f)
    Wo = np.asarray(inputs["Wo"], f)
    ctab = _count_table()
    in_maps = []

    for core in range(NCORES):
        b, hg = core // 2, core % 2
        sl = slice(hg * D, (hg + 1) * D)
        cs = comb_sorted[b]
        # reference window per (sorted) token: clip(ceil(comb), 1, 20)
        Tj = np.clip(np.ceil(cs), 1, T_MAX).astype(np.int64)
        packf = np.zeros((128, PACKW), f)
        # c(m, T_j) per token; BIGF where m > T_j
        cfull = ctab.T[Tj - 1, :]                     # [S, 20]
        for i in range(4):
            packf[:, 20 * i:20 * (i + 1)] = cfull[128 * i:128 * (i + 1), :]
        packc = np.zeros((1, PACKC), f)
        packc[0, PC_COMB:PC_COMB + S] = cs
        packc[0, PC_ONES:PC_ONES + 128] = 1.0
        packr = np.zeros((E, PACKR), f)
        packr[:, R_X:R_X + S] = x[b][perm[b]].T
        packr[:, R_WV:R_WV + D] = Wv[:, sl]
        packr[:, R_WQ:R_WQ + D] = Wq[:, sl]
        packr[:, R_WK:R_WK + D] = Wk[:, sl]
        packb = np.zeros((128, 1024), f)
        for hp in range(2):
            packb[:, 512 * hp:512 * (hp + 1)] = \
                Wo[hg * D + 128 * hp:hg * D + 128 * (hp + 1), :]
        in_maps.append({"packf": packf, "packc": packc, "packr": packr,
                        "packb": packb.astype(bf)})
    return in_maps


def kernel(**inputs):
    global last_exec_ns, last_results
    f = np.float32
    x = np.asarray(inputs["x"], f)
    bo = np.asarray(inputs["bo"], f)

    comb20 = _host_comb20(x,
                          np.asarray(inputs["g1"], f), np.asarray(inputs["gb1"], f),
                          np.asarray(inputs["g2"], f), np.asarray(inputs["gb2"], f),
                          np.asarray(inputs["g3"], f), np.asarray(inputs["gb3"], f),
                          np.asarray(inputs["c1"], f), np.asarray(inputs["cb1"], f),
                          np.asarray(inputs["c2"], f), np.asarray(inputs["cb2"], f))
    perm, comb_sorted, tsteps, A, mask_needed, Amin = _plan(comb20)

    key = (tsteps, tuple(A), tuple(mask_needed), tuple(Amin))
    if key not in _BUILD_CACHE:
        _BUILD_CACHE[key] = _build(key)
    nc = _BUILD_CACHE[key]

    in_maps = make_in_maps(inputs, perm, comb_sorted, tsteps)

    from concourse.bass_utils import run_bass_kernel_spmd
    trace = bool(int(os.environ.get("KERNEL_TRACE", "0")))
    try:
        res = run_bass_kernel_spmd(nc, in_maps, core_ids=list(range(NCORES)),
                                   trace=trace)
    except (ModuleNotFoundError, ImportError):
        res = run_bass_kernel_spmd(nc, in_maps, core_ids=list(range(NCORES)),
                                   trace=False)
    last_results = res
    last_exec_ns = res.exec_time_ns

    out = np.empty((B, S, E), np.float32)
    for b in range(B):
        inv = np.empty(S, np.int64)
        inv[perm[b]] = np.arange(S)
        part = res.results[2 * b]["out"] + res.results[2 * b + 1]["out"]
        out[b] = part[inv] + bo[None, :]
    return out
